# revision 9
# baseline (speedup 1.0000x reference)
"""Causal GQA self-attention (B=2, T=2048, C=2048, 16 heads / 4 KV groups,
head_size=128, RoPE) on 8 Trainium2 NeuronCores.

Sharding: tensor-parallel over the 4 KV groups x data-parallel over the 2
batch elements -> 8 cores, core = b*4 + g. Each core computes its group's
QKV projection, RoPE, causal SDPA for the group's 4 query heads, and the
partial output projection (w_proj input-dim shard). The proj partials are
reduced on the host (equivalent of the post-proj all-reduce).

v2 schedule:
- Softmax row sums no longer ride a ones-matmul on the PE per j-block.
  Exp strips accumulate into an SBUF tile (DVE + GpSimd adds), and the
  128-partition reduction is one gpsimd.partition_all_reduce per
  (quarter, head). Frees ~25us of TensorEngine time.
- V^T -> V transposes moved from PE to the DMA xbar (dma_start_transpose).
- QKV chains (t4-major), attention quarters, and the partial output
  projection are interleaved in program order: attention on quarter q
  runs with the t4=q-1 QKV chains and quarter q+1's projection as PE
  filler, so the ACT-bound exp stream never idles the PE.
- PSUM: qkv 2 + scores 2 + y 2 + proj 2 banks, double-buffered everywhere.
- x chunk 0 is loaded as four 512-column quarters so the first QKV matmul
  starts as soon as ~330KB (not 11MB) has landed.
"""

import sys
import math

for _p in ("/opt/trn_rl_repo", "/root/.axon_site/_ro/trn_rl_repo"):
    if _p not in sys.path:
        sys.path.insert(0, _p)

import numpy as np
import ml_dtypes

import concourse.bass as bass  # noqa: F401  (registers engine classes)
import concourse.bacc as bacc
import concourse.tile as tile
from concourse import mybir
from concourse import bass_isa
from concourse.bass_utils import run_bass_kernel_spmd
from contextlib import ExitStack

BF16 = ml_dtypes.bfloat16
P = 128
T = 2048
C = 2048
NT = T // P        # 16 t-blocks
NCC = C // P       # 16 contraction chunks
NQ = 4             # query heads per core
FQKV = 6 * P       # 768
FY = NQ * P        # 512
SCALE = 1.0 / math.sqrt(P)
NEG = -1.0e30

dt = mybir.dt
AF = mybir.ActivationFunctionType
ALU = mybir.AluOpType
RED = bass_isa.ReduceOp

TRACE = False
_CACHE = {}


def _build():
    nc = bacc.Bacc("TRN2", target_bir_lowering=False, debug=False, num_devices=8)
    xT_d = nc.dram_tensor("xT", [C, T], dt.bfloat16, kind="ExternalInput").ap()
    wqkT_d = nc.dram_tensor("wqkT", [C, FQKV], dt.bfloat16, kind="ExternalInput").ap()
    wpT_d = nc.dram_tensor("wpT", [FY, T], dt.bfloat16, kind="ExternalInput").ap()
    cosT_d = nc.dram_tensor("cosT", [P, T], dt.float32, kind="ExternalInput").ap()
    sinS_d = nc.dram_tensor("sinS", [P, T], dt.float32, kind="ExternalInput").ap()
    out_d = nc.dram_tensor("out", [T, C], dt.float32, kind="ExternalOutput").ap()

    with tile.TileContext(nc) as tc, ExitStack() as ctx:
        const = ctx.enter_context(tc.tile_pool(name="const", bufs=1))
        # causal mask for the diagonal 128x128 block of scores^T:
        # element (p=j, f=i): keep 0 where i - j >= 0, else -1e30
        maskf = const.tile([P, P], dt.float32, tag="mask", name="maskf")
        nc.gpsimd.memset(maskf, 0.0)
        nc.gpsimd.affine_select(
            out=maskf, in_=maskf, compare_op=ALU.is_ge, fill=NEG,
            base=0, pattern=[[1, P]], channel_multiplier=-1,
        )

        trig = ctx.enter_context(tc.tile_pool(name="trig", bufs=1))
        cosT = trig.tile([P, T], dt.float32, tag="cos", name="cosT")
        sinS = trig.tile([P, T], dt.float32, tag="sin", name="sinS")

        persist = ctx.enter_context(tc.tile_pool(name="persist", bufs=1))
        qrot = [persist.tile([P, T], dt.bfloat16, tag=f"q{h}", name=f"q{h}") for h in range(NQ)]
        krot = persist.tile([P, T], dt.bfloat16, tag="k", name="krot")
        vraw = persist.tile([P, T], dt.bfloat16, tag="vr", name="vraw")   # v^T (d-major)
        vt = persist.tile([P, T], dt.bfloat16, tag="vt", name="vt")       # v t-major blocks
        y_sb = [persist.tile([P, T], dt.bfloat16, tag=f"y{h}", name=f"ysb{h}") for h in range(NQ)]
        wp_t = [persist.tile([P, T], dt.bfloat16, tag=f"wp{j}", name=f"wp{j}") for j in range(NQ)]

        # ---- input DMAs ------------------------------------------------
        xw_pool = ctx.enter_context(tc.tile_pool(name="xw", bufs=1))
        wt, xt = [], []
        x0q = []
        for ci in range(NCC):
            tw = xw_pool.tile([P, FQKV], dt.bfloat16, tag=f"w{ci}", name=f"wt{ci}")
            wt.append(tw)
            if ci == 0:
                for t4 in range(4):
                    tq = xw_pool.tile([P, 512], dt.bfloat16, tag=f"x0q{t4}", name=f"x0q{t4}")
                    x0q.append(tq)
                xt.append(None)
            else:
                tx = xw_pool.tile([P, T], dt.bfloat16, tag=f"x{ci}", name=f"xt{ci}")
                xt.append(tx)

        def x_sl(ci, t4):
            if ci == 0:
                return x0q[t4]
            return xt[ci][:, t4 * 512:(t4 + 1) * 512]

        nc.sync.dma_start(wt[0], wqkT_d[0:P, :])
        for t4 in range(4):
            nc.sync.dma_start(x0q[t4], xT_d[0:P, t4 * 512:(t4 + 1) * 512])
        for ci in range(1, NCC):
            nc.sync.dma_start(wt[ci], wqkT_d[ci * P:(ci + 1) * P, :])
            nc.sync.dma_start(xt[ci], xT_d[ci * P:(ci + 1) * P, :])
            if ci == 3:
                nc.sync.dma_start(cosT, cosT_d)
                nc.sync.dma_start(sinS, sinS_d)
            if ci == 8:
                for j in range(NQ):
                    nc.sync.dma_start(wp_t[j], wpT_d[j * P:(j + 1) * P, :])

        # ---- pools -----------------------------------------------------
        rtmp = ctx.enter_context(tc.tile_pool(name="rtmp", bufs=3))
        strip_pool = ctx.enter_context(tc.tile_pool(name="strip", bufs=4))
        esa_pool = ctx.enter_context(tc.tile_pool(name="esa", bufs=4))
        esb_pool = ctx.enter_context(tc.tile_pool(name="esb", bufs=2))
        rcp_pool = ctx.enter_context(tc.tile_pool(name="rcp", bufs=2))
        ostage = ctx.enter_context(tc.tile_pool(name="ostage", bufs=2))

        # ---- emission helpers ------------------------------------------
        def qkv_chain(f, t4, pool):
            """One [128,512] QKV chain: accumulate over the 16 C-chunks,
            then RoPE (q/k) into the rotated SBUF tile."""
            ps = pool.tile([P, 512], dt.float32, tag="qkv", name="qkvps_t")
            st = slice(t4 * 512, (t4 + 1) * 512)
            for ci in range(NCC):
                nc.tensor.matmul(
                    ps,
                    lhsT=wt[ci][:, f * P:(f + 1) * P],
                    rhs=x_sl(ci, t4),
                    start=(ci == 0), stop=(ci == NCC - 1),
                )
            if f == 5:
                # v: copy to SBUF d-major, then DMA-xbar transpose the four
                # [128,128] blocks into t-major vt.
                nc.scalar.copy(vraw[:, st], ps)
                for b in range(4):
                    tb = t4 * 4 + b
                    bs = slice(tb * P, (tb + 1) * P)
                    nc.sync.dma_start_transpose(vt[:, bs], vraw[:, bs])
            else:
                dest = qrot[f] if f < NQ else krot
                t1 = rtmp.tile([P, 512], dt.float32, tag="r1", name="ropet1")
                nc.vector.tensor_mul(t1, ps, cosT[:, st])
                t2 = rtmp.tile([P, 512], dt.float32, tag="r2", name="ropet2")
                nc.vector.tensor_mul(t2[0:64, :], ps[64:128, :], sinS[0:64, st])
                nc.vector.tensor_mul(t2[64:128, :], ps[0:64, :], sinS[64:128, st])
                nc.gpsimd.tensor_add(dest[:, st], t1, t2)

        def attn_quarter(q, fillers):
            """Attention for the 512-wide i-window q, heads 0..3.

            Per j-block: scores matmul -> (mask) -> exp -> y matmul, with the
            scores matmul software-pipelined one block ahead of the y matmul.
            Row sums accumulate on DVE/GpSimd into SBUF, reduced across
            partitions by gpsimd.partition_all_reduce. `fillers` are PE work
            closures (QKV chains / proj chunks) injected between heads and
            spread through the jb stream to cover the exp latency."""
            q_lo = q * 512
            njb = 4 * q + 4
            fill = list(fillers)
            total_jb = njb * NQ
            fill_every = max(1, total_jb // (len(fill) + 1)) if fill else 0
            jb_ctr = [0]
            for h in range(NQ):
                qT = qrot[h]
                yps = ypsp.tile([P, 512], dt.float32, tag="y", name="ypst")
                esa = esa_pool.tile([P, 512], dt.float32, tag="esa", name="esat")
                esb = esb_pool.tile([P, 512], dt.float32, tag="esb", name="esbt")
                nb_used = False
                pend = None  # (jb, strip, w, c0) waiting for its y matmul

                def emit_y(jb, strip, w, c0):
                    nonlocal nb_used
                    j_sl = slice(jb * P, (jb + 1) * P)
                    nc.tensor.matmul(
                        yps[:, c0:], lhsT=vt[:, j_sl], rhs=strip[:, :w],
                        start=(jb == 0), stop=(jb == njb - 1),
                    )
                    # esum accumulation (DVE mostly, every 4th full block on
                    # GpSimd to shorten the serial RMW chain)
                    on_b = (0 < jb < 4 * q) and (jb % 4 == 2)
                    if jb == 0:
                        nc.vector.tensor_copy(esa, strip)
                    elif on_b:
                        if not nb_used:
                            nc.gpsimd.tensor_copy(esb, strip)
                        else:
                            nc.gpsimd.tensor_add(esb, esb, strip)
                    else:
                        eng = nc.vector
                        eng.tensor_add(esa[:, c0:], esa[:, c0:], strip[:, :w])
                    if on_b:
                        nb_used = True

                for jb in range(njb):
                    j_sl = slice(jb * P, (jb + 1) * P)
                    i_lo = max(jb * P, q_lo)
                    w = q_lo + 512 - i_lo
                    c0 = 512 - w
                    strip = strip_pool.tile([P, 512], dt.bfloat16, tag="strip", name="stript")
                    ps = scps.tile([P, 512], dt.float32, tag="sc", name="scpst")
                    nc.tensor.matmul(
                        ps[:, :w], lhsT=krot[:, j_sl], rhs=qT[:, i_lo:i_lo + w],
                        start=True, stop=True,
                    )
                    if jb >= 4 * q:  # diagonal block: apply causal mask
                        nc.vector.tensor_add(ps[:, :P], ps[:, :P], maskf)
                    nc.scalar.activation(strip[:, :w], ps[:, :w], AF.Exp, scale=SCALE)
                    if pend is not None:
                        emit_y(*pend)
                    pend = (jb, strip, w, c0)
                    jb_ctr[0] += 1
                    if fill and fill_every and jb_ctr[0] % fill_every == 0:
                        fill.pop(0)()
                emit_y(*pend)
                # row sums: fold the gpsimd partial in, then reduce across
                # partitions and normalize.
                if nb_used:
                    nc.vector.tensor_add(esa, esa, esb)
                nc.gpsimd.partition_all_reduce(esb, esa, channels=P, reduce_op=RED.add)
                rcp = rcp_pool.tile([P, 512], dt.float32, tag="rcp", name="rcpt")
                nc.vector.reciprocal_approx_fast(out=rcp, in_=esb)
                nc.vector.tensor_mul(y_sb[h][:, q_lo:q_lo + 512], yps, rcp)
            # drain leftover fillers
            for f_ in fill:
                f_()

        _cp = [0]

        def proj_chunk(tb, oh):
            """Partial output projection for t-block tb, 1024-wide output
            half oh: two 512-col PSUM chunks, staged copy, one DMA."""
            t_sl = slice(tb * P, (tb + 1) * P)
            ot = ostage.tile([P, 1024], dt.float32, tag="o", name="otile")
            for o2 in range(2):
                pp = prps.tile([P, 512], dt.float32, tag="pr", name="prpst")
                o_lo = oh * 1024 + o2 * 512
                for f4 in range(NQ):
                    nc.tensor.matmul(
                        pp,
                        lhsT=y_sb[f4][:, t_sl],
                        rhs=wp_t[f4][:, o_lo:o_lo + 512],
                        start=(f4 == 0), stop=(f4 == NQ - 1),
                    )
                eng = (nc.scalar.copy, nc.vector.tensor_copy)[_cp[0] % 2]
                _cp[0] += 1
                eng(ot[:, o2 * 512:(o2 + 1) * 512], pp)
            nc.sync.dma_start(out_d[t_sl, oh * 1024:(oh + 1) * 1024], ot)

        def proj_quarter_fillers(q):
            return [
                (lambda tb=tb, oh=oh: proj_chunk(tb, oh))
                for tb in range(4 * q, 4 * q + 4)
                for oh in range(2)
            ]

        def chain_fillers(t4):
            return [(lambda f=f: qkv_chain(f, t4, qkvps)) for f in range(4)]

        # ---- program ---------------------------------------------------
        # Load-phase chains get a wide (8-bank) PSUM pool so the PE has
        # enough open accumulation chains while x streams in; it closes
        # before the attention-phase pools take the banks over.
        with tc.tile_pool(name="qkvld", bufs=8, space="PSUM") as qkv_ld:
            for t4 in range(4):
                qkv_chain(4, t4, qkv_ld)        # k
            for t4 in range(4):
                qkv_chain(5, t4, qkv_ld)        # v (+ xbar transposes)
            for f in range(4):
                qkv_chain(f, 3, qkv_ld)         # q heads, top quarter first
        qkvps = ctx.enter_context(tc.tile_pool(name="qkvps", bufs=2, space="PSUM"))
        scps = ctx.enter_context(tc.tile_pool(name="scps", bufs=2, space="PSUM"))
        ypsp = ctx.enter_context(tc.tile_pool(name="ypsp", bufs=2, space="PSUM"))
        prps = ctx.enter_context(tc.tile_pool(name="prps", bufs=2, space="PSUM"))
        attn_quarter(3, chain_fillers(2))
        attn_quarter(2, chain_fillers(1) + proj_quarter_fillers(3))
        attn_quarter(1, chain_fillers(0) + proj_quarter_fillers(2))
        attn_quarter(0, proj_quarter_fillers(1))
        for fl in proj_quarter_fillers(0):
            fl()

    nc.compile()
    return nc


def kernel(x, w_attn, w_proj, cos, sin):
    x = np.asarray(x, dtype=np.float32)
    w_attn = np.asarray(w_attn, dtype=np.float32)
    w_proj = np.asarray(w_proj, dtype=np.float32)
    cos = np.asarray(cos, dtype=np.float32)
    sin = np.asarray(sin, dtype=np.float32)

    if "nc" not in _CACHE:
        _CACHE["nc"] = _build()
    nc = _CACHE["nc"]

    cosT = np.ascontiguousarray(cos.T)                      # [128, T] f32
    sinT = np.ascontiguousarray(sin.T)
    sinS = sinT.copy()
    sinS[:64] = -sinS[:64]

    in_maps = []
    for core in range(8):
        b, g = core // 4, core % 4
        xT = np.ascontiguousarray(x[b].T).astype(BF16)                        # [C, T]
        wqkT = np.ascontiguousarray(w_attn[g * FQKV:(g + 1) * FQKV].T).astype(BF16)  # [C, 768]
        wpT = np.ascontiguousarray(w_proj[:, g * FY:(g + 1) * FY].T).astype(BF16)    # [512, T]
        in_maps.append({"xT": xT, "wqkT": wqkT, "wpT": wpT, "cosT": cosT, "sinS": sinS})

    res = run_bass_kernel_spmd(nc, in_maps, core_ids=list(range(8)), trace=TRACE)
    if TRACE:
        _CACHE["last_results"] = res

    out = np.zeros((2, T, C), dtype=np.float32)
    for core in range(8):
        b = core // 4
        out[b] += res.results[core]["out"]
    return out


# revision 10
# speedup vs baseline: 1.4794x; 1.4794x over previous
"""Causal GQA self-attention (B=2, T=2048, C=2048, 16 heads / 4 KV groups,
head_size=128, RoPE) on 8 Trainium2 NeuronCores.

Sharding: tensor-parallel over the 4 KV groups x data-parallel over the 2
batch elements -> 8 cores, core = b*4 + g. Each core computes its group's
QKV projection, RoPE, causal SDPA for the group's 4 query heads, and the
partial output projection (w_proj input-dim shard). The proj partials are
reduced on the host (equivalent of the post-proj all-reduce).

All matmuls run in bf16 with fp32 PSUM accumulation. Inputs are transposed
and cast to bf16 on the host so every DMA is a contiguous, layout-perfect
load (contraction dims land on SBUF partitions).

Schedule notes: k/v are projected first so attention overlaps the q QKV
tail; attention is quarter-major over 512-wide i-windows (largest quarter
first) with uniform [128,512] PSUM tiles, and the partial output
projection is interleaved per quarter. Row sums ride a ones-matmul on the
TensorEngine (broadcast to all partitions), softmax normalize uses
reciprocal_approx_fast + multiply on DVE, RoPE adds run on idle GpSimd,
and DMAs are ordered so the first QKV accumulation chain starts
immediately.
"""

import sys
import math

for _p in ("/opt/trn_rl_repo", "/root/.axon_site/_ro/trn_rl_repo"):
    if _p not in sys.path:
        sys.path.insert(0, _p)

import numpy as np
import ml_dtypes

import concourse.bass as bass  # noqa: F401  (registers engine classes)
import concourse.bacc as bacc
import concourse.tile as tile
from concourse import mybir
from concourse.bass_utils import run_bass_kernel_spmd
from concourse.masks import make_identity
from contextlib import ExitStack

BF16 = ml_dtypes.bfloat16
P = 128
T = 2048
C = 2048
NT = T // P        # 16 t-blocks
NCC = C // P       # 16 contraction chunks
NF = 6             # f-blocks per core: q0..q3, k, v
NQ = 4             # query heads per core
FQKV = NF * P      # 768
FY = NQ * P        # 512
SCALE = 1.0 / math.sqrt(P)
NEG = -1.0e30

dt = mybir.dt
AF = mybir.ActivationFunctionType
ALU = mybir.AluOpType

TRACE = False
_CACHE = {}


def _build():
    nc = bacc.Bacc("TRN2", target_bir_lowering=False, debug=False, num_devices=8)
    xT_d = nc.dram_tensor("xT", [C, T], dt.bfloat16, kind="ExternalInput").ap()
    wqkT_d = nc.dram_tensor("wqkT", [C, FQKV], dt.bfloat16, kind="ExternalInput").ap()
    wpT_d = nc.dram_tensor("wpT", [FY, T], dt.bfloat16, kind="ExternalInput").ap()
    cosT_d = nc.dram_tensor("cosT", [P, T], dt.float32, kind="ExternalInput").ap()
    sinS_d = nc.dram_tensor("sinS", [P, T], dt.float32, kind="ExternalInput").ap()
    out_d = nc.dram_tensor("out", [T, C], dt.float32, kind="ExternalOutput").ap()

    with tile.TileContext(nc) as tc, ExitStack() as ctx:
        const = ctx.enter_context(tc.tile_pool(name="const", bufs=1))
        identity = const.tile([P, P], dt.bfloat16, tag="id", name="identity")
        make_identity(nc, identity)
        ones_bf = const.tile([P, P], dt.bfloat16, tag="ones", name="ones_bf")
        nc.gpsimd.memset(ones_bf, 1.0)
        # causal mask for the diagonal 128x128 block of scores^T:
        # element (p=j, f=i): keep 0 where i - j >= 0, else -1e30
        maskf = const.tile([P, P], dt.float32, tag="mask", name="maskf")
        nc.gpsimd.memset(maskf, 0.0)
        nc.gpsimd.affine_select(
            out=maskf, in_=maskf, compare_op=ALU.is_ge, fill=NEG,
            base=0, pattern=[[1, P]], channel_multiplier=-1,
        )

        trig = ctx.enter_context(tc.tile_pool(name="trig", bufs=1))
        cosT = trig.tile([P, T], dt.float32, tag="cos", name="cosT")
        sinS = trig.tile([P, T], dt.float32, tag="sin", name="sinS")

        persist = ctx.enter_context(tc.tile_pool(name="persist", bufs=1))
        qrot = [persist.tile([P, T], dt.bfloat16, tag=f"q{h}", name=f"q{h}") for h in range(NQ)]
        krot = persist.tile([P, T], dt.bfloat16, tag="k", name="krot")
        vraw = persist.tile([P, T], dt.bfloat16, tag="vr", name="vraw")   # v^T (d-major)
        vt = persist.tile([P, T], dt.bfloat16, tag="vt", name="vt")       # v t-major blocks
        y_sb = [persist.tile([P, T], dt.bfloat16, tag=f"y{h}", name=f"ysb{h}") for h in range(NQ)]
        wp_t = [persist.tile([P, T], dt.bfloat16, tag=f"wp{j}", name=f"wp{j}") for j in range(NQ)]

        # DMA order matters for the pipeline head: interleave w/x chunk pairs
        # so the first accumulation chain can start immediately; everything
        # not needed until RoPE / proj loads afterwards.
        xw_pool = ctx.enter_context(tc.tile_pool(name="xw", bufs=1))
        xt, wt = [], []
        for ci in range(NCC):
            tw = xw_pool.tile([P, FQKV], dt.bfloat16, tag=f"w{ci}", name=f"wt{ci}")
            nc.sync.dma_start(tw, wqkT_d[ci * P:(ci + 1) * P, :])
            wt.append(tw)
            tx = xw_pool.tile([P, T], dt.bfloat16, tag=f"x{ci}", name=f"xt{ci}")
            nc.sync.dma_start(tx, xT_d[ci * P:(ci + 1) * P, :])
            xt.append(tx)
        nc.sync.dma_start(cosT, cosT_d)
        nc.sync.dma_start(sinS, sinS_d)
        for j in range(NQ):
            nc.sync.dma_start(wp_t[j], wpT_d[j * P:(j + 1) * P, :])

        # ---------------- Phase 1: QKV^T = wqkT.T @ xT, fused RoPE ----------
        # k and v first so attention can start while q1..q3 still project.
        with tc.tile_pool(name="rtmp", bufs=6) as rtmp, \
             tc.tile_pool(name="qkvps", bufs=7, space="PSUM") as qkvps, \
             tc.tile_pool(name="vtps", bufs=1, space="PSUM") as vtps:
            for f in (4, 5, 0, 1, 2, 3):
                for t4 in (3, 2, 1, 0):  # 512-wide t quarters, one PSUM bank each
                    ps = qkvps.tile([P, 512], dt.float32, tag="qkv", name="qkvps_t")
                    st = slice(t4 * 512, (t4 + 1) * 512)
                    for ci in range(NCC):
                        nc.tensor.matmul(
                            ps,
                            lhsT=wt[ci][:, f * P:(f + 1) * P],
                            rhs=xt[ci][:, st],
                            start=(ci == 0), stop=(ci == NCC - 1),
                        )
                    if f != 5:
                        # RoPE (rotate-halves) in fp32, write bf16
                        dest = qrot[f] if f < NQ else krot
                        t1 = rtmp.tile([P, 512], dt.float32, tag="r1", name="ropet1")
                        nc.vector.tensor_mul(t1, ps, cosT[:, st])
                        t2 = rtmp.tile([P, 512], dt.float32, tag="r2", name="ropet2")
                        nc.vector.tensor_mul(t2[0:64, :], ps[64:128, :], sinS[0:64, st])
                        nc.vector.tensor_mul(t2[64:128, :], ps[0:64, :], sinS[64:128, st])
                        nc.gpsimd.tensor_add(dest[:, st], t1, t2)
                    else:
                        nc.any.tensor_copy(vraw[:, st], ps)
                if f == 5:
                    # v^T -> v (t-major [j-part, d]) via PE transpose
                    for tb in range(NT):
                        pst = vtps.tile([P, P], dt.bfloat16, tag="vtp", name="vtpst")
                        nc.tensor.transpose(pst, vraw[:, tb * P:(tb + 1) * P], identity)
                        nc.any.tensor_copy(vt[:, tb * P:(tb + 1) * P], pst)

        # ------------- Phase 2: attention + interleaved partial proj --------
        # Quarter-major: for each 512-wide i-window, run all 4 heads' causal
        # attention (scores^T chunks [j-part, i-free], ACT exp, y^T and
        # broadcast row-sums via PE), then immediately project those 4
        # t-blocks. Uniform [128,512] PSUM tiles keep all pools in 8 banks.
        with tc.tile_pool(name="strip", bufs=8) as strip_pool, \
             tc.tile_pool(name="ssb", bufs=3) as ssb_pool, \
             tc.tile_pool(name="ostage", bufs=4) as ostage, \
             tc.tile_pool(name="scps", bufs=3, space="PSUM") as scps, \
             tc.tile_pool(name="ypsp", bufs=2, space="PSUM") as ypsp, \
             tc.tile_pool(name="spsp", bufs=1, space="PSUM") as spsp, \
             tc.tile_pool(name="prps", bufs=1, space="PSUM") as prps:
            for q in (3, 2, 1, 0):
                q_lo = q * 512
                for h in range(NQ):
                    qT = qrot[h]
                    yps = ypsp.tile([P, 512], dt.float32, tag="y", name="ypst")
                    sps = spsp.tile([P, 512], dt.float32, tag="s", name="spst")
                    njb = 4 * q + 4
                    for jb in range(njb):
                        j_sl = slice(jb * P, (jb + 1) * P)
                        i_lo = max(jb * P, q_lo)
                        w = q_lo + 512 - i_lo
                        c0 = 512 - w  # column offset inside the 512 window
                        strip = strip_pool.tile([P, 512], dt.bfloat16, tag="strip", name="stript")
                        ps = scps.tile([P, 512], dt.float32, tag="sc", name="scpst")
                        nc.tensor.matmul(
                            ps[:, :w], lhsT=krot[:, j_sl], rhs=qT[:, i_lo:i_lo + w],
                            start=True, stop=True,
                        )
                        if jb >= 4 * q:  # diagonal block: apply causal mask
                            nc.vector.tensor_add(ps[:, :P], ps[:, :P], maskf)
                        nc.scalar.activation(strip[:, :w], ps[:, :w], AF.Exp, scale=SCALE)
                        st_flag = (jb == 0)
                        sp_flag = (jb == njb - 1)
                        nc.tensor.matmul(
                            yps[:, c0:], lhsT=vt[:, j_sl], rhs=strip[:, :w],
                            start=st_flag, stop=sp_flag,
                        )
                        nc.tensor.matmul(
                            sps[:, c0:], lhsT=ones_bf, rhs=strip[:, :w],
                            start=st_flag, stop=sp_flag,
                        )
                    # normalize: y * (1/rowsum) (sums broadcast on all partitions)
                    rcp = ssb_pool.tile([P, 512], dt.float32, tag="ssb", name="rcpt")
                    nc.vector.reciprocal_approx_fast(out=rcp, in_=sps)
                    nc.vector.tensor_mul(y_sb[h][:, q_lo:q_lo + 512], yps, rcp)
                # partial proj for this quarter's 4 t-blocks
                for tb in range(4 * q, 4 * q + 4):
                    t_sl = slice(tb * P, (tb + 1) * P)
                    for oh in range(2):
                        pp = prps.tile([P, 1024], dt.float32, tag="pr", name="prpst")
                        for f4 in range(NQ):
                            for o2 in range(2):
                                o_lo = oh * 1024 + o2 * 512
                                nc.tensor.matmul(
                                    pp[:, o2 * 512:(o2 + 1) * 512],
                                    lhsT=y_sb[f4][:, t_sl],
                                    rhs=wp_t[f4][:, o_lo:o_lo + 512],
                                    start=(f4 == 0), stop=(f4 == NQ - 1),
                                )
                        ot = ostage.tile([P, 1024], dt.float32, tag="o", name="otile")
                        nc.any.tensor_copy(ot, pp)
                        nc.sync.dma_start(out_d[t_sl, oh * 1024:(oh + 1) * 1024], ot)

    nc.compile()
    return nc


def kernel(x, w_attn, w_proj, cos, sin):
    x = np.asarray(x, dtype=np.float32)
    w_attn = np.asarray(w_attn, dtype=np.float32)
    w_proj = np.asarray(w_proj, dtype=np.float32)
    cos = np.asarray(cos, dtype=np.float32)
    sin = np.asarray(sin, dtype=np.float32)

    if "nc" not in _CACHE:
        _CACHE["nc"] = _build()
    nc = _CACHE["nc"]

    cosT = np.ascontiguousarray(cos.T)                      # [128, T] f32
    sinT = np.ascontiguousarray(sin.T)
    sinS = sinT.copy()
    sinS[:64] = -sinS[:64]

    in_maps = []
    for core in range(8):
        b, g = core // 4, core % 4
        xT = np.ascontiguousarray(x[b].T).astype(BF16)                        # [C, T]
        wqkT = np.ascontiguousarray(w_attn[g * FQKV:(g + 1) * FQKV].T).astype(BF16)  # [C, 768]
        wpT = np.ascontiguousarray(w_proj[:, g * FY:(g + 1) * FY].T).astype(BF16)    # [512, T]
        in_maps.append({"xT": xT, "wqkT": wqkT, "wpT": wpT, "cosT": cosT, "sinS": sinS})

    res = run_bass_kernel_spmd(nc, in_maps, core_ids=list(range(8)), trace=TRACE)
    if TRACE:
        _CACHE["last_results"] = res

    out = np.zeros((2, T, C), dtype=np.float32)
    for core in range(8):
        b = core // 4
        out[b] += res.results[core]["out"]
    return out

